# revision 6
# baseline (speedup 1.0000x reference)
"""Causal self-attention (B=2, S=4096, D=512, H=8) on 8 Trainium2 NeuronCores.

Sharding: tensor-parallel over heads. Core h computes head h for both batch
elements: QKV projections for its head, causal flash attention, and its
partial o_proj contribution y_h = attn_out_h @ Wo[h*64:(h+1)*64, :]. The 8
partial [8192, 512] outputs are summed on the host (plus bo).

Per-core layout (hd = 64, S = 4096, tokens per batch on 32 k-tiles of 128):
  - xT [512, 8192] (host-pretransposed x) streams in as [128f, 4c, 512t]
    tiles; QK projection matmul (lhsT = [Wq_h | Wk_h] chunk) produces
    psum [Q.T; K.T] per 512-token block; V.T separately, then PE-transposed
    to V natural.
  - QT2 [128, 4096]/batch: Q.T duplicated in both partition halves (dup via
    SBUF->SBUF DMA). KT2 [128, 2048]/batch: even k-tiles in partitions 0:64,
    odd in 64:128. This feeds 2-packed S.T matmuls (K=64 row-tiling: pair
    of k-tiles computed concurrently at array rows 0-63 / 64-127).
  - S.T chunk = 4 k-tiles -> psum [128, 4, 512] (4 banks); one ACT exp call
    [128, 2048] PSUM->SBUF produces P.T (float32r); diagonal chunks get a
    0/1 causal mask multiply on DVE.
  - AV: U'[65, 512] += V'_kt.T @ P.T_kt where V' = [V | ones]; row 64 = L
    (softmax denominators). L row is PE-transposed to a column, DVE
    reciprocal -> invL, applied as per-partition scale on the o_proj output.
  - y tile [128q, 512] = U.T @ Wo_h, scaled by invL, DMA'd out.

All matmuls run in float32r (TF32-like, ~1.6e-4 rel err, 1 cycle/column).
"""

import sys

for _p in ("/opt/trn_rl_repo", "/root/.axon_site/_ro/trn_rl_repo"):
    if _p not in sys.path:
        sys.path.insert(0, _p)

import numpy as np

import concourse.bass as bass
import concourse.mybir as mybir
import concourse.tile as tile
from concourse import bacc
from concourse.bass_utils import run_bass_kernel_spmd

B = 2
S = 4096
D = 512
H = 8
HD = 64
TOK = B * S          # 8192
NTB = TOK // 512     # 16 token blocks of 512
NKT = S // 128       # 32 k-tiles per batch
NQB = S // 512       # 8 q-blocks per batch
SCALE = HD ** -0.5

F32 = mybir.dt.float32
F32R = mybir.dt.float32r

_CACHE = {}


def _build():
    nc = bacc.Bacc("TRN2", target_bir_lowering=False, debug=False, num_devices=8)

    xt_d = nc.dram_tensor("xt", [D, TOK], F32R, kind="ExternalInput")
    wqk_d = nc.dram_tensor("wqk", [D, 128], F32R, kind="ExternalInput")
    wv_d = nc.dram_tensor("wv", [D, HD], F32R, kind="ExternalInput")
    wo_d = nc.dram_tensor("wo", [HD, D], F32R, kind="ExternalInput")
    bqk_d = nc.dram_tensor("bqk", [128, 1], F32, kind="ExternalInput")
    bv_d = nc.dram_tensor("bv", [HD, 1], F32, kind="ExternalInput")
    mask_d = nc.dram_tensor("mask", [128, 4, 512], F32R, kind="ExternalInput")
    ident_d = nc.dram_tensor("ident", [128, 128], F32R, kind="ExternalInput")
    ones_d = nc.dram_tensor("ones32", [128, NKT], F32R, kind="ExternalInput")
    y_d = nc.dram_tensor("y", [TOK, D], F32, kind="ExternalOutput")

    xt_r = xt_d.ap().rearrange("(c p) t -> p c t", p=128)      # [128, 4, 8192]
    wqk_r = wqk_d.ap().rearrange("(c p) m -> p c m", p=128)    # [128, 4, 128]
    wv_r = wv_d.ap().rearrange("(c p) m -> p c m", p=128)      # [128, 4, 64]

    with tile.TileContext(nc) as tc:
        import contextlib

        with contextlib.ExitStack() as ctx:
            singles = ctx.enter_context(tc.tile_pool(name="singles", bufs=1))
            xpool = ctx.enter_context(tc.tile_pool(name="xt", bufs=3))
            ptpool = ctx.enter_context(tc.tile_pool(name="pt", bufs=2))
            upool = ctx.enter_context(tc.tile_pool(name="usb", bufs=2))
            ypool = ctx.enter_context(tc.tile_pool(name="ysb", bufs=3))
            smpool = ctx.enter_context(tc.tile_pool(name="small", bufs=3))
            kstpool = ctx.enter_context(tc.tile_pool(name="kst", bufs=2))

            ps_st = ctx.enter_context(tc.tile_pool(name="ps_st", bufs=1, space="PSUM"))
            ps_u = ctx.enter_context(tc.tile_pool(name="ps_u", bufs=2, space="PSUM"))
            ps_misc = ctx.enter_context(
                tc.tile_pool(name="ps_misc", bufs=2, space="PSUM")
            )

            # --- constants / weights -----------------------------------
            wqk_sb = singles.tile([128, 4, 128], F32R)
            wv_sb = singles.tile([128, 4, HD], F32R)
            wo_sb = singles.tile([HD, D], F32R)
            bqk_sb = singles.tile([128, 1], F32)
            bv_sb = singles.tile([HD, 1], F32)
            mask_sb = singles.tile([128, 4, 512], F32R)
            ident = singles.tile([128, 128], F32R)
            ones_sb = singles.tile([128, NKT], F32R)
            nc.sync.dma_start(out=wqk_sb, in_=wqk_r)
            nc.sync.dma_start(out=wv_sb, in_=wv_r)
            nc.sync.dma_start(out=wo_sb, in_=wo_d.ap())
            nc.sync.dma_start(out=bqk_sb, in_=bqk_d.ap())
            nc.sync.dma_start(out=bv_sb, in_=bv_d.ap())
            nc.sync.dma_start(out=mask_sb, in_=mask_d.ap())
            nc.sync.dma_start(out=ident, in_=ident_d.ap())
            nc.sync.dma_start(out=ones_sb, in_=ones_d.ap())

            # --- persistent per-batch activation buffers ---------------
            # QT2: Q.T duplicated in both halves; KT2: even k-tiles top,
            # odd bottom; V': [V | 1] per k-tile ([128, 65] blocks).
            qt2 = [singles.tile([128, S], F32R, tag=f"qt2_{b}", name=f"qt2_{b}") for b in range(B)]
            kt2 = [singles.tile([128, S // 2], F32R, tag=f"kt2_{b}", name=f"kt2_{b}") for b in range(B)]
            vp = [singles.tile([128, NKT * 65], F32R, tag=f"vp_{b}", name=f"vp_{b}") for b in range(B)]
            for b in range(B):
                nc.sync.dma_start(
                    out=vp[b].rearrange("p (t c) -> p t c", c=65)[:, :, 64:65],
                    in_=ones_d.ap().rearrange("p (t c) -> p t c", c=1),
                )

            def proj_block(b, tb):
                """Projections for 512 tokens (block tb of batch b)."""
                t0 = b * S + tb * 512
                xt_sb = xpool.tile([128, 4, 512], F32R, tag="xt")
                nc.sync.dma_start(out=xt_sb, in_=xt_r[:, :, t0 : t0 + 512])

                qk_ps = ps_misc.tile([128, 512], F32, tag="m")
                for c in range(4):
                    nc.tensor.matmul(
                        qk_ps,
                        wqk_sb[:, c, :],
                        xt_sb[:, c, :],
                        start=(c == 0),
                        stop=(c == 3),
                    )
                vt_ps = ps_misc.tile([128, 512], F32, tag="m")
                for c in range(4):
                    nc.tensor.matmul(
                        vt_ps[0:HD, :],
                        wv_sb[:, c, :],
                        xt_sb[:, c, :],
                        start=(c == 0),
                        stop=(c == 3),
                    )

                # Q.T -> QT2 top half (+bias), then DMA-dup to bottom half
                cols = slice(tb * 512, (tb + 1) * 512)
                nc.vector.tensor_scalar_add(
                    qt2[b][0:64, cols], qk_ps[0:64, :], bqk_sb[0:64, 0:1]
                )
                nc.sync.dma_start(out=qt2[b][64:128, cols], in_=qt2[b][0:64, cols])

                # K.T tiles: psum rows 64:128 hold K.T for 4 k-tiles.
                # Odd tiles go straight to KT2 bottom half (lane-aligned);
                # even tiles are staged (+bias) then DMA'd to the top half.
                kcols = slice(tb * 256, (tb + 1) * 256)
                qk_k = qk_ps[64:128, :].rearrange("p (t j) -> p t j", t=4)
                nc.vector.tensor_scalar_add(
                    kt2[b][64:128, kcols].rearrange("p (t j) -> p t j", t=2),
                    qk_k[:, 1::2, :],
                    bqk_sb[64:128, 0:1],
                )
                kstage = kstpool.tile([128, 2, 128], F32R, tag="kst")
                nc.vector.tensor_scalar_add(
                    kstage[64:128, :, :], qk_k[:, 0::2, :], bqk_sb[64:128, 0:1]
                )
                nc.sync.dma_start(
                    out=kt2[b][0:64, kcols], in_=kstage[64:128, :, :]
                )

                # V.T (+bias) -> transpose to V natural -> V' blocks
                vt_sb = kstpool.tile([HD, 512], F32R, tag="vt")
                nc.vector.tensor_scalar_add(vt_sb, vt_ps[0:HD, :], bv_sb[:, 0:1])
                for j in range(4):
                    kt = tb * 4 + j
                    vtr_ps = ps_misc.tile([128, HD], F32R, tag="m")
                    nc.tensor.transpose(
                        vtr_ps, vt_sb[:, j * 128 : (j + 1) * 128], ident[0:64, 0:64]
                    )
                    nc.vector.tensor_copy(
                        vp[b][:, kt * 65 : kt * 65 + 64], vtr_ps
                    )

            def attn_qblock(b, qb):
                """Attention + o_proj for q-block qb (512 queries) of batch b."""
                q0 = qb * 512
                u_ps = ps_u.tile([65, 512], F32, tag="u")
                for j in range(qb + 1):  # chunks of 4 k-tiles
                    st = ps_st.tile([128, 4, 512], F32, tag="st")
                    for i2 in range(2):  # k-tile pairs, 2-packed on PE
                        blk = 2 * j + i2
                        nc.tensor.matmul(
                            st[:, 2 * i2, :],
                            kt2[b][0:64, blk * 128 : (blk + 1) * 128],
                            qt2[b][0:64, q0 : q0 + 512],
                            start=True,
                            stop=True,
                        )
                        nc.tensor.matmul(
                            st[:, 2 * i2 + 1, :],
                            kt2[b][64:128, blk * 128 : (blk + 1) * 128],
                            qt2[b][64:128, q0 : q0 + 512],
                            start=True,
                            stop=True,
                        )
                    pt = ptpool.tile([128, 4, 512], F32R, tag="pt")
                    nc.scalar.activation(
                        pt, st, mybir.ActivationFunctionType.Exp, scale=SCALE
                    )
                    if j == qb:  # diagonal chunk: causal mask
                        nc.vector.tensor_mul(pt, pt, mask_sb)
                    for j2 in range(4):
                        kt = 4 * j + j2
                        nc.tensor.matmul(
                            u_ps,
                            vp[b][:, kt * 65 : kt * 65 + 65],
                            pt[:, j2, :],
                            start=(j == 0 and j2 == 0),
                            stop=(j == qb and j2 == 3),
                            skip_group_check=True,
                        )

                u_sb = upool.tile([65, 512], F32R, tag="u")
                nc.vector.tensor_copy(u_sb, u_ps)
                # L (row 64) -> column layout via PE transpose, reciprocal
                ltr_ps = ps_misc.tile([128, 8], F32R, tag="m")
                for j2 in range(4):
                    # [128, 2] dst keeps fp32r even-count ISA rule happy
                    nc.tensor.transpose(
                        ltr_ps[:, 2 * j2 : 2 * j2 + 2],
                        u_sb[64:65, j2 * 128 : (j2 + 1) * 128],
                        ones_sb[64:65, 0:2],
                    )
                invl = smpool.tile([128, 8], F32, tag="invl")
                nc.vector.reciprocal(invl, ltr_ps)

                for j2 in range(4):
                    y_ps = ps_misc.tile([128, 512], F32, tag="m")
                    nc.tensor.matmul(
                        y_ps,
                        u_sb[0:64, j2 * 128 : (j2 + 1) * 128],
                        wo_sb,
                        start=True,
                        stop=True,
                    )
                    y_sb = ypool.tile([128, 512], F32, tag="y")
                    nc.vector.tensor_scalar_mul(
                        y_sb, y_ps, invl[:, 2 * j2 : 2 * j2 + 1]
                    )
                    r0 = b * S + q0 + j2 * 128
                    nc.sync.dma_start(out=y_d.ap()[r0 : r0 + 128, :], in_=y_sb)

            # Pipeline: proj(tb) immediately enables attn(qb=tb).
            for b in range(B):
                for tb in range(8):
                    proj_block(b, tb)
                    attn_qblock(b, tb)

    nc.compile()
    return nc


def _prep_inputs(x, Wq, bq, Wk, bk, Wv, bv, Wo, bo):
    xt = np.ascontiguousarray(x.reshape(TOK, D).T).astype(np.float32)
    mask = np.zeros((128, 4, 512), dtype=np.float32)
    for d in range(4):
        p = np.arange(128)[:, None]
        c = np.arange(512)[None, :]
        mask[:, d, :] = (p + 128 * d <= c).astype(np.float32)

    in_maps = []
    for h in range(H):
        hs = slice(h * HD, (h + 1) * HD)
        in_maps.append(
            {
                "xt": xt,
                "wqk": np.ascontiguousarray(
                    np.concatenate([Wq[:, hs], Wk[:, hs]], axis=1)
                ).astype(np.float32),
                "wv": np.ascontiguousarray(Wv[:, hs]).astype(np.float32),
                "wo": np.ascontiguousarray(Wo[hs, :]).astype(np.float32),
                "bqk": np.concatenate([bq[hs], bk[hs]]).reshape(128, 1).astype(
                    np.float32
                ),
                "bv": bv[hs].reshape(HD, 1).astype(np.float32),
                "mask": mask,
                "ident": np.eye(128, dtype=np.float32),
                "ones32": np.ones((128, NKT), dtype=np.float32),
            }
        )
    return in_maps


def _install_ntff_hook():
    """Register the axon NTFF profiling hook (test-only plumbing)."""
    import types

    try:
        from antenv.axon_hooks import set_axon_ntff_profile_hook  # noqa: F401
    except ImportError:
        m = types.ModuleType("antenv.axon_hooks")
        m._HOOK = None
        m.set_axon_ntff_profile_hook = lambda h: setattr(m, "_HOOK", h)
        m.get_axon_ntff_profile_hook = lambda: m._HOOK
        sys.modules["antenv.axon_hooks"] = m
        import antenv

        antenv.axon_hooks = m
    from antenv.axon_hooks import (
        get_axon_ntff_profile_hook,
        set_axon_ntff_profile_hook,
    )

    if get_axon_ntff_profile_hook() is None:
        import trn_agent_boot.trn_boot as tb

        set_axon_ntff_profile_hook(
            tb._ntff_profile_via_ctypes("/opt/axon/libaxon_pjrt.so")
        )


def kernel(x, Wq, bq, Wk, bk, Wv, bv, Wo, bo, _trace=False):
    x, Wq, bq, Wk, bk, Wv, bv, Wo, bo = (
        np.asarray(a, dtype=np.float32) for a in (x, Wq, bq, Wk, bk, Wv, bv, Wo, bo)
    )
    if "nc" not in _CACHE:
        _CACHE["nc"] = _build()
    nc = _CACHE["nc"]
    in_maps = _prep_inputs(x, Wq, bq, Wk, bk, Wv, bv, Wo, bo)
    kwargs = {}
    if _trace:
        _install_ntff_hook()
        kwargs = dict(trace=True, trace_cores=[0])
    res = run_bass_kernel_spmd(nc, in_maps, core_ids=list(range(8)), **kwargs)
    _CACHE["last_result"] = res
    y = np.zeros((TOK, D), dtype=np.float32)
    for r in res.results:
        y += r["y"]
    y += bo[None, :]
    return y.reshape(B, S, D)


# revision 9
# speedup vs baseline: 1.0510x; 1.0510x over previous
"""Causal self-attention (B=2, S=4096, D=512, H=8) on 8 Trainium2 NeuronCores.

Sharding: tensor-parallel over heads. Core h computes head h for both batch
elements: QKV projections for its head, causal flash attention, and its
partial o_proj contribution y_h = attn_out_h @ Wo[h*64:(h+1)*64, :]. The 8
partial [8192, 512] outputs are summed on the host (plus bo).

Per-core layout (hd = 64, S = 4096, tokens per batch on 32 k-tiles of 128):
  - xT [512, 8192] (host-pretransposed x) streams in as [128f, 4c, 512t]
    tiles; QK projection matmul (lhsT = [Wq_h | Wk_h] chunk) produces
    psum [Q.T; K.T] per 512-token block; V.T separately, then PE-transposed
    to V natural.
  - QT2 [128, 4096]/batch: Q.T duplicated in both partition halves (dup via
    SBUF->SBUF DMA). KT2 [128, 2048]/batch: even k-tiles in partitions 0:64,
    odd in 64:128. This feeds 2-packed S.T matmuls (K=64 row-tiling: pair
    of k-tiles computed concurrently at array rows 0-63 / 64-127).
  - S.T chunk = 4 k-tiles -> psum [128, 4, 512] (4 banks); one ACT exp call
    [128, 2048] PSUM->SBUF produces P.T (float32r); diagonal chunks get a
    0/1 causal mask multiply on DVE.
  - AV: U'[65, 512] += V'_kt.T @ P.T_kt where V' = [V | ones]; row 64 = L
    (softmax denominators). L row is PE-transposed to a column, DVE
    reciprocal -> invL, applied as per-partition scale on the o_proj output.
  - y tile [128q, 512] = U.T @ Wo_h, scaled by invL, DMA'd out.

All matmuls run in float32r (TF32-like, ~1.6e-4 rel err, 1 cycle/column).
"""

import sys

for _p in ("/opt/trn_rl_repo", "/root/.axon_site/_ro/trn_rl_repo"):
    if _p not in sys.path:
        sys.path.insert(0, _p)

import numpy as np

import concourse.bass as bass
import concourse.mybir as mybir
import concourse.tile as tile
from concourse import bacc
from concourse.bass_utils import run_bass_kernel_spmd

B = 2
S = 4096
D = 512
H = 8
HD = 64
TOK = B * S          # 8192
NTB = TOK // 512     # 16 token blocks of 512
NKT = S // 128       # 32 k-tiles per batch
NQB = S // 512       # 8 q-blocks per batch
SCALE = HD ** -0.5

F32 = mybir.dt.float32
F32R = mybir.dt.float32r

_CACHE = {}


def _build():
    nc = bacc.Bacc("TRN2", target_bir_lowering=False, debug=False, num_devices=8)

    xt_d = nc.dram_tensor("xt", [D, TOK], F32R, kind="ExternalInput")
    wqk_d = nc.dram_tensor("wqk", [D, 128], F32R, kind="ExternalInput")
    wv_d = nc.dram_tensor("wv", [D, HD], F32R, kind="ExternalInput")
    wo_d = nc.dram_tensor("wo", [HD, D], F32R, kind="ExternalInput")
    bqk_d = nc.dram_tensor("bqk", [128, 1], F32, kind="ExternalInput")
    bv_d = nc.dram_tensor("bv", [HD, 1], F32, kind="ExternalInput")
    mask_d = nc.dram_tensor("mask", [128, 4, 512], F32R, kind="ExternalInput")
    ident_d = nc.dram_tensor("ident", [128, 128], F32R, kind="ExternalInput")
    ones_d = nc.dram_tensor("ones32", [128, NKT], F32R, kind="ExternalInput")
    y_d = nc.dram_tensor("y", [TOK, D], F32, kind="ExternalOutput")

    xt_r = xt_d.ap().rearrange("(c p) t -> p c t", p=128)      # [128, 4, 8192]
    wqk_r = wqk_d.ap().rearrange("(c p) m -> p c m", p=128)    # [128, 4, 128]
    wv_r = wv_d.ap().rearrange("(c p) m -> p c m", p=128)      # [128, 4, 64]

    with tile.TileContext(nc) as tc:
        import contextlib

        with contextlib.ExitStack() as ctx:
            singles = ctx.enter_context(tc.tile_pool(name="singles", bufs=1))
            xpool = ctx.enter_context(tc.tile_pool(name="xt", bufs=3))
            ptpool = ctx.enter_context(tc.tile_pool(name="pt", bufs=3))
            upool = ctx.enter_context(tc.tile_pool(name="usb", bufs=2))
            ypool = ctx.enter_context(tc.tile_pool(name="ysb", bufs=3))
            smpool = ctx.enter_context(tc.tile_pool(name="small", bufs=3))
            kstpool = ctx.enter_context(tc.tile_pool(name="kst", bufs=2))

            ps_st = ctx.enter_context(tc.tile_pool(name="ps_st", bufs=2, space="PSUM"))
            ps_u = ctx.enter_context(tc.tile_pool(name="ps_u", bufs=2, space="PSUM"))
            ps_misc = ctx.enter_context(
                tc.tile_pool(name="ps_misc", bufs=2, space="PSUM")
            )

            # --- constants / weights -----------------------------------
            wqk_sb = singles.tile([128, 4, 128], F32R)
            wv_sb = singles.tile([128, 4, HD], F32R)
            wo_sb = singles.tile([HD, D], F32R)
            bqk_sb = singles.tile([128, 1], F32)
            bv_sb = singles.tile([HD, 1], F32)
            mask_sb = singles.tile([128, 4, 512], F32R)
            ident = singles.tile([128, 128], F32R)
            ones_sb = singles.tile([128, NKT], F32R)
            nc.sync.dma_start(out=wqk_sb, in_=wqk_r)
            nc.sync.dma_start(out=wv_sb, in_=wv_r)
            nc.sync.dma_start(out=wo_sb, in_=wo_d.ap())
            nc.sync.dma_start(out=bqk_sb, in_=bqk_d.ap())
            nc.sync.dma_start(out=bv_sb, in_=bv_d.ap())
            nc.sync.dma_start(out=mask_sb, in_=mask_d.ap())
            nc.sync.dma_start(out=ident, in_=ident_d.ap())
            nc.sync.dma_start(out=ones_sb, in_=ones_d.ap())

            # --- persistent per-batch activation buffers ---------------
            # QT2: Q.T duplicated in both halves; KT2: even k-tiles top,
            # odd bottom; V': [V | 1] per k-tile ([128, 65] blocks).
            qt2 = [singles.tile([128, S], F32R, tag=f"qt2_{b}", name=f"qt2_{b}") for b in range(B)]
            kt2 = [singles.tile([128, S // 2], F32R, tag=f"kt2_{b}", name=f"kt2_{b}") for b in range(B)]
            vp = [singles.tile([128, NKT * 65], F32R, tag=f"vp_{b}", name=f"vp_{b}") for b in range(B)]
            for b in range(B):
                nc.sync.dma_start(
                    out=vp[b].rearrange("p (t c) -> p t c", c=65)[:, :, 64:65],
                    in_=ones_d.ap().rearrange("p (t c) -> p t c", c=1),
                )

            def proj_block(b, tb):
                """Projections for 512 tokens (block tb of batch b)."""
                t0 = b * S + tb * 512
                xt_sb = xpool.tile([128, 4, 512], F32R, tag="xt")
                nc.sync.dma_start(out=xt_sb, in_=xt_r[:, :, t0 : t0 + 512])

                qk_ps = ps_misc.tile([128, 512], F32, tag="m")
                for c in range(4):
                    nc.tensor.matmul(
                        qk_ps,
                        wqk_sb[:, c, :],
                        xt_sb[:, c, :],
                        start=(c == 0),
                        stop=(c == 3),
                    )
                vt_ps = ps_misc.tile([128, 512], F32, tag="m")
                for c in range(4):
                    nc.tensor.matmul(
                        vt_ps[0:HD, :],
                        wv_sb[:, c, :],
                        xt_sb[:, c, :],
                        start=(c == 0),
                        stop=(c == 3),
                    )

                # Q.T -> QT2 top half (+bias), then DMA-dup to bottom half
                cols = slice(tb * 512, (tb + 1) * 512)
                nc.vector.tensor_scalar_add(
                    qt2[b][0:64, cols], qk_ps[0:64, :], bqk_sb[0:64, 0:1]
                )
                nc.sync.dma_start(out=qt2[b][64:128, cols], in_=qt2[b][0:64, cols])

                # K.T tiles: psum rows 64:128 hold K.T for 4 k-tiles.
                # Odd tiles go straight to KT2 bottom half (lane-aligned);
                # even tiles are staged (+bias) then DMA'd to the top half.
                kcols = slice(tb * 256, (tb + 1) * 256)
                qk_k = qk_ps[64:128, :].rearrange("p (t j) -> p t j", t=4)
                nc.vector.tensor_scalar_add(
                    kt2[b][64:128, kcols].rearrange("p (t j) -> p t j", t=2),
                    qk_k[:, 1::2, :],
                    bqk_sb[64:128, 0:1],
                )
                kstage = kstpool.tile([128, 2, 128], F32R, tag="kst")
                nc.vector.tensor_scalar_add(
                    kstage[64:128, :, :], qk_k[:, 0::2, :], bqk_sb[64:128, 0:1]
                )
                nc.sync.dma_start(
                    out=kt2[b][0:64, kcols], in_=kstage[64:128, :, :]
                )

                # V.T (+bias) -> transpose to V natural -> V' blocks
                vt_sb = kstpool.tile([HD, 512], F32R, tag="vt")
                nc.vector.tensor_scalar_add(vt_sb, vt_ps[0:HD, :], bv_sb[:, 0:1])
                for j in range(4):
                    kt = tb * 4 + j
                    vtr_ps = ps_misc.tile([128, HD], F32R, tag="m")
                    nc.tensor.transpose(
                        vtr_ps, vt_sb[:, j * 128 : (j + 1) * 128], ident[0:64, 0:64]
                    )
                    nc.vector.tensor_copy(
                        vp[b][:, kt * 65 : kt * 65 + 64], vtr_ps
                    )

            def attn_qblock(b, qb):
                """Attention + o_proj for q-block qb (512 queries) of batch b."""
                q0 = qb * 512
                u_ps = ps_u.tile([65, 512], F32, tag="u")
                n_chunks = 2 * (qb + 1)  # chunks of 2 k-tiles, double-buffered
                for j in range(n_chunks):
                    st = ps_st.tile([128, 2, 512], F32, tag="st")
                    nc.tensor.matmul(
                        st[:, 0, :],
                        kt2[b][0:64, j * 128 : (j + 1) * 128],
                        qt2[b][0:64, q0 : q0 + 512],
                        start=True,
                        stop=True,
                    )
                    nc.tensor.matmul(
                        st[:, 1, :],
                        kt2[b][64:128, j * 128 : (j + 1) * 128],
                        qt2[b][64:128, q0 : q0 + 512],
                        start=True,
                        stop=True,
                    )
                    pt = ptpool.tile([128, 2, 512], F32R, tag="pt")
                    nc.scalar.activation(
                        pt, st, mybir.ActivationFunctionType.Exp, scale=SCALE
                    )
                    if j >= n_chunks - 2:  # diagonal chunks: causal mask
                        d0 = (j % 2) * 2
                        nc.vector.tensor_mul(pt, pt, mask_sb[:, d0 : d0 + 2, :])
                    for j2 in range(2):
                        kt = 2 * j + j2
                        nc.tensor.matmul(
                            u_ps,
                            vp[b][:, kt * 65 : kt * 65 + 65],
                            pt[:, j2, :],
                            start=(j == 0 and j2 == 0),
                            stop=(j == n_chunks - 1 and j2 == 1),
                            skip_group_check=True,
                        )

                u_sb = upool.tile([65, 512], F32R, tag="u")
                nc.vector.tensor_copy(u_sb, u_ps)
                # L (row 64) -> column layout via PE transpose, reciprocal
                ltr_ps = ps_misc.tile([128, 8], F32R, tag="m")
                for j2 in range(4):
                    # [128, 2] dst keeps fp32r even-count ISA rule happy
                    nc.tensor.transpose(
                        ltr_ps[:, 2 * j2 : 2 * j2 + 2],
                        u_sb[64:65, j2 * 128 : (j2 + 1) * 128],
                        ones_sb[64:65, 0:2],
                    )
                invl = smpool.tile([128, 8], F32, tag="invl")
                nc.vector.reciprocal(invl, ltr_ps)

                for j2 in range(4):
                    y_ps = ps_misc.tile([128, 512], F32, tag="m")
                    nc.tensor.matmul(
                        y_ps,
                        u_sb[0:64, j2 * 128 : (j2 + 1) * 128],
                        wo_sb,
                        start=True,
                        stop=True,
                    )
                    y_sb = ypool.tile([128, 512], F32, tag="y")
                    nc.vector.tensor_scalar_mul(
                        y_sb, y_ps, invl[:, 2 * j2 : 2 * j2 + 1]
                    )
                    r0 = b * S + q0 + j2 * 128
                    nc.sync.dma_start(out=y_d.ap()[r0 : r0 + 128, :], in_=y_sb)

            # Pipeline: proj(tb) immediately enables attn(qb=tb).
            for b in range(B):
                for tb in range(8):
                    proj_block(b, tb)
                    attn_qblock(b, tb)

    nc.compile()
    return nc


def _prep_inputs(x, Wq, bq, Wk, bk, Wv, bv, Wo, bo):
    xt = np.ascontiguousarray(x.reshape(TOK, D).T).astype(np.float32)
    mask = np.zeros((128, 4, 512), dtype=np.float32)
    for d in range(4):
        p = np.arange(128)[:, None]
        c = np.arange(512)[None, :]
        mask[:, d, :] = (p + 128 * d <= c).astype(np.float32)

    in_maps = []
    for h in range(H):
        hs = slice(h * HD, (h + 1) * HD)
        in_maps.append(
            {
                "xt": xt,
                "wqk": np.ascontiguousarray(
                    np.concatenate([Wq[:, hs], Wk[:, hs]], axis=1)
                ).astype(np.float32),
                "wv": np.ascontiguousarray(Wv[:, hs]).astype(np.float32),
                "wo": np.ascontiguousarray(Wo[hs, :]).astype(np.float32),
                "bqk": np.concatenate([bq[hs], bk[hs]]).reshape(128, 1).astype(
                    np.float32
                ),
                "bv": bv[hs].reshape(HD, 1).astype(np.float32),
                "mask": mask,
                "ident": np.eye(128, dtype=np.float32),
                "ones32": np.ones((128, NKT), dtype=np.float32),
            }
        )
    return in_maps


def _install_ntff_hook():
    """Register the axon NTFF profiling hook (test-only plumbing)."""
    import types

    try:
        from antenv.axon_hooks import set_axon_ntff_profile_hook  # noqa: F401
    except ImportError:
        m = types.ModuleType("antenv.axon_hooks")
        m._HOOK = None
        m.set_axon_ntff_profile_hook = lambda h: setattr(m, "_HOOK", h)
        m.get_axon_ntff_profile_hook = lambda: m._HOOK
        sys.modules["antenv.axon_hooks"] = m
        import antenv

        antenv.axon_hooks = m
    from antenv.axon_hooks import (
        get_axon_ntff_profile_hook,
        set_axon_ntff_profile_hook,
    )

    if get_axon_ntff_profile_hook() is None:
        import trn_agent_boot.trn_boot as tb

        set_axon_ntff_profile_hook(
            tb._ntff_profile_via_ctypes("/opt/axon/libaxon_pjrt.so")
        )


def kernel(x, Wq, bq, Wk, bk, Wv, bv, Wo, bo, _trace=False):
    x, Wq, bq, Wk, bk, Wv, bv, Wo, bo = (
        np.asarray(a, dtype=np.float32) for a in (x, Wq, bq, Wk, bk, Wv, bv, Wo, bo)
    )
    if "nc" not in _CACHE:
        _CACHE["nc"] = _build()
    nc = _CACHE["nc"]
    in_maps = _prep_inputs(x, Wq, bq, Wk, bk, Wv, bv, Wo, bo)
    kwargs = {}
    if _trace:
        _install_ntff_hook()
        kwargs = dict(trace=True, trace_cores=[0])
    res = run_bass_kernel_spmd(nc, in_maps, core_ids=list(range(8)), **kwargs)
    _CACHE["last_result"] = res
    y = np.zeros((TOK, D), dtype=np.float32)
    for r in res.results:
        y += r["y"]
    y += bo[None, :]
    return y.reshape(B, S, D)


# revision 11
# speedup vs baseline: 1.3803x; 1.3133x over previous
"""Causal self-attention (B=2, S=4096, D=512, H=8) on 8 Trainium2 NeuronCores.

Sharding: tensor-parallel over heads. Core h computes head h for both batch
elements: QKV projections for its head, causal flash attention, and its
partial (unnormalized) o_proj contribution y_h = U_h @ Wo[h*64:(h+1)*64, :]
plus the per-query softmax denominators L_h. The host computes
sum_h(y_h / L_h) + bo.

Per-core layout (hd = 64, S = 4096, 32 k-tiles of 128 per batch):
  - xT [512, 8192] (host-pretransposed x) streams in as [128f, 4c, 512t]
    tiles; QK projection matmul (lhsT = [Wq_h | Wk_h] chunk) produces
    psum [Q.T; K.T] per 512-token block; V.T separately, then PE-transposed
    to V natural (bf16).
  - QT2 [128, 4096]/batch: Q.T duplicated in both partition halves (dup via
    SBUF->SBUF DMA). KT2 [128, 2048]/batch: even k-tiles in partitions 0:64,
    odd in 64:128. S.T matmul pairs (K=64) are emitted back-to-back so the
    PE packs them onto array row-groups 0-63 / 64-127 concurrently.
  - S.T chunk = 2 k-tiles -> psum [128, 2, 512] (double-buffered); one ACT
    exp call [128, 1024] PSUM->SBUF produces P.T in bf16; the two diagonal
    chunks of each q-block get a 0/1 causal mask multiply on DVE.
  - AV (bf16): U'[65, 512] += V'_kt.T @ P.T_kt with V' = [V | ones]; row 64
    accumulates L. AV for chunk j is emitted after the S.T pair of chunk
    j+1 to keep the pair adjacent in the PE stream.
  - y tiles [128q, 512] = U.T @ Wo_h (fp32r, q-subtile pairs packed via a
    DMA-duplicated U and host-duplicated Wo), DMA'd out unnormalized with L.

Matmuls: scores/projections/o_proj in float32r (~1.6e-4), AV in bf16.
"""

import sys

for _p in ("/opt/trn_rl_repo", "/root/.axon_site/_ro/trn_rl_repo"):
    if _p not in sys.path:
        sys.path.insert(0, _p)

import numpy as np

import concourse.bass as bass
import concourse.mybir as mybir
import concourse.tile as tile
from concourse import bacc
from concourse.bass_utils import run_bass_kernel_spmd

B = 2
S = 4096
D = 512
H = 8
HD = 64
TOK = B * S          # 8192
NKT = S // 128       # 32 k-tiles per batch
SCALE = HD ** -0.5

F32 = mybir.dt.float32
F32R = mybir.dt.float32r
BF16 = mybir.dt.bfloat16

_CACHE = {}


def _build():
    nc = bacc.Bacc("TRN2", target_bir_lowering=False, debug=False, num_devices=8)

    xt_d = nc.dram_tensor("xt", [D, TOK], F32R, kind="ExternalInput")
    wqk_d = nc.dram_tensor("wqk", [D, 128], F32R, kind="ExternalInput")
    wv_d = nc.dram_tensor("wv", [D, HD], F32R, kind="ExternalInput")
    wo_d = nc.dram_tensor("wo", [128, D], F32R, kind="ExternalInput")
    bqk_d = nc.dram_tensor("bqk", [128, 1], F32, kind="ExternalInput")
    bv_d = nc.dram_tensor("bv", [HD, 1], F32, kind="ExternalInput")
    mask_d = nc.dram_tensor("mask", [128, 4, 512], BF16, kind="ExternalInput")
    identb_d = nc.dram_tensor("identb", [64, 64], BF16, kind="ExternalInput")
    onesb_d = nc.dram_tensor("onesb", [128, NKT], BF16, kind="ExternalInput")
    y_d = nc.dram_tensor("y", [TOK, D], F32, kind="ExternalOutput")
    l_d = nc.dram_tensor("l", [TOK], F32R, kind="ExternalOutput")

    xt_r = xt_d.ap().rearrange("(c p) t -> p c t", p=128)      # [128, 4, 8192]
    wqk_r = wqk_d.ap().rearrange("(c p) m -> p c m", p=128)    # [128, 4, 128]
    wv_r = wv_d.ap().rearrange("(c p) m -> p c m", p=128)      # [128, 4, 64]

    with tile.TileContext(nc) as tc:
        import contextlib

        with contextlib.ExitStack() as ctx:
            singles = ctx.enter_context(tc.tile_pool(name="singles", bufs=1))
            xpool = ctx.enter_context(tc.tile_pool(name="xt", bufs=3))
            ptpool = ctx.enter_context(tc.tile_pool(name="pt", bufs=4))
            upool = ctx.enter_context(tc.tile_pool(name="usb", bufs=2))
            ypool = ctx.enter_context(tc.tile_pool(name="ysb", bufs=4))
            kstpool = ctx.enter_context(tc.tile_pool(name="kst", bufs=2))

            ps_st = ctx.enter_context(
                tc.tile_pool(name="ps_st", bufs=2, space="PSUM")
            )
            ps_u = ctx.enter_context(tc.tile_pool(name="ps_u", bufs=2, space="PSUM"))
            ps_misc = ctx.enter_context(
                tc.tile_pool(name="ps_misc", bufs=2, space="PSUM")
            )

            # --- constants / weights -----------------------------------
            wqk_sb = singles.tile([128, 4, 128], F32R)
            wv_sb = singles.tile([128, 4, HD], F32R)
            wo_sb = singles.tile([128, D], F32R)
            bqk_sb = singles.tile([128, 1], F32)
            bv_sb = singles.tile([HD, 1], F32)
            mask_sb = singles.tile([128, 4, 512], BF16)
            identb = singles.tile([64, 64], BF16)
            nc.sync.dma_start(out=wqk_sb, in_=wqk_r)
            nc.sync.dma_start(out=wv_sb, in_=wv_r)
            nc.sync.dma_start(out=wo_sb, in_=wo_d.ap())
            nc.sync.dma_start(out=bqk_sb, in_=bqk_d.ap())
            nc.sync.dma_start(out=bv_sb, in_=bv_d.ap())
            nc.sync.dma_start(out=mask_sb, in_=mask_d.ap())
            nc.sync.dma_start(out=identb, in_=identb_d.ap())

            # --- persistent per-batch activation buffers ---------------
            qt2 = [
                singles.tile([128, S], F32R, tag=f"qt2_{b}", name=f"qt2_{b}")
                for b in range(B)
            ]
            kt2 = [
                singles.tile([128, S // 2], F32R, tag=f"kt2_{b}", name=f"kt2_{b}")
                for b in range(B)
            ]
            vp = [
                singles.tile([128, NKT * 65], BF16, tag=f"vp_{b}", name=f"vp_{b}")
                for b in range(B)
            ]
            for b in range(B):
                nc.sync.dma_start(
                    out=vp[b].rearrange("p (t c) -> p t c", c=65)[:, :, 64:65],
                    in_=onesb_d.ap().rearrange("p (t c) -> p t c", c=1),
                )

            def proj_block(b, tb):
                """Projections for 512 tokens (block tb of batch b)."""
                t0 = b * S + tb * 512
                xt_sb = xpool.tile([128, 4, 512], F32R, tag="xt")
                nc.sync.dma_start(out=xt_sb, in_=xt_r[:, :, t0 : t0 + 512])

                qk_ps = ps_misc.tile([128, 512], F32, tag="m")
                for c in range(4):
                    nc.tensor.matmul(
                        qk_ps,
                        wqk_sb[:, c, :],
                        xt_sb[:, c, :],
                        start=(c == 0),
                        stop=(c == 3),
                    )
                vt_ps = ps_misc.tile([128, 512], F32, tag="m")
                for c in range(4):
                    nc.tensor.matmul(
                        vt_ps[0:HD, :],
                        wv_sb[:, c, :],
                        xt_sb[:, c, :],
                        start=(c == 0),
                        stop=(c == 3),
                    )

                # Q.T -> QT2 top half (+bias), then DMA-dup to bottom half
                cols = slice(tb * 512, (tb + 1) * 512)
                nc.vector.tensor_scalar_add(
                    qt2[b][0:64, cols], qk_ps[0:64, :], bqk_sb[0:64, 0:1]
                )
                nc.sync.dma_start(out=qt2[b][64:128, cols], in_=qt2[b][0:64, cols])

                # K.T tiles: odd tiles lane-aligned to KT2 bottom half; even
                # tiles staged (+bias) then DMA'd to the top half.
                kcols = slice(tb * 256, (tb + 1) * 256)
                qk_k = qk_ps[64:128, :].rearrange("p (t j) -> p t j", t=4)
                nc.vector.tensor_scalar_add(
                    kt2[b][64:128, kcols].rearrange("p (t j) -> p t j", t=2),
                    qk_k[:, 1::2, :],
                    bqk_sb[64:128, 0:1],
                )
                kstage = kstpool.tile([128, 2, 128], F32R, tag="kst")
                nc.vector.tensor_scalar_add(
                    kstage[64:128, :, :], qk_k[:, 0::2, :], bqk_sb[64:128, 0:1]
                )
                nc.sync.dma_start(out=kt2[b][0:64, kcols], in_=kstage[64:128, :, :])

                # V.T (+bias, bf16) -> PE transpose to V natural -> V' blocks
                vt_sb = kstpool.tile([HD, 512], BF16, tag="vt")
                nc.vector.tensor_scalar_add(vt_sb, vt_ps[0:HD, :], bv_sb[:, 0:1])
                for j in range(4):
                    kt = tb * 4 + j
                    vtr_ps = ps_misc.tile([128, HD], BF16, tag="m")
                    nc.tensor.transpose(
                        vtr_ps, vt_sb[:, j * 128 : (j + 1) * 128], identb
                    )
                    nc.vector.tensor_copy(vp[b][:, kt * 65 : kt * 65 + 64], vtr_ps)

            def attn_qblock(b, qb):
                """Attention + unnormalized o_proj for q-block qb of batch b."""
                q0 = qb * 512
                u_ps = ps_u.tile([65, 512], F32, tag="u")
                n_chunks = 2 * (qb + 1)  # chunks of 2 k-tiles

                def emit_av(pt, j):
                    for j2 in range(2):
                        kt = 2 * j + j2
                        nc.tensor.matmul(
                            u_ps,
                            vp[b][:, kt * 65 : kt * 65 + 65],
                            pt[:, j2, :],
                            start=(kt == 0),
                            stop=(kt == 2 * n_chunks - 1),
                            skip_group_check=True,
                        )

                prev_pt = None
                for j in range(n_chunks):
                    st = ps_st.tile([128, 2, 512], F32, tag="st")
                    # S.T pair: emitted back-to-back -> PE row-group packing
                    nc.tensor.matmul(
                        st[:, 0, :],
                        kt2[b][0:64, j * 128 : (j + 1) * 128],
                        qt2[b][0:64, q0 : q0 + 512],
                        start=True,
                        stop=True,
                    )
                    nc.tensor.matmul(
                        st[:, 1, :],
                        kt2[b][64:128, j * 128 : (j + 1) * 128],
                        qt2[b][64:128, q0 : q0 + 512],
                        start=True,
                        stop=True,
                    )
                    pt = ptpool.tile([128, 2, 512], BF16, tag="pt")
                    nc.scalar.activation(
                        pt, st, mybir.ActivationFunctionType.Exp, scale=SCALE
                    )
                    if j >= n_chunks - 2:  # diagonal chunks: causal mask
                        d0 = (j % 2) * 2
                        nc.vector.tensor_mul(pt, pt, mask_sb[:, d0 : d0 + 2, :])
                    if prev_pt is not None:
                        emit_av(prev_pt, j - 1)
                    prev_pt = pt
                emit_av(prev_pt, n_chunks - 1)

                # U' -> SBUF; dup U into partitions 64:128 for y-pair packing
                u_sb = upool.tile([65, 512], F32R, tag="u")
                nc.vector.tensor_copy(u_sb, u_ps)
                u2_sb = upool.tile([128, 512], F32R, tag="u2")
                nc.sync.dma_start(out=u2_sb[64:128, :], in_=u_sb[0:64, :])

                row0 = b * S + q0
                nc.sync.dma_start(
                    out=l_d.ap()[row0 : row0 + 512].rearrange("(p c) -> p c", p=1),
                    in_=u_sb[64:65, :],
                )

                # y = U.T @ Wo_h (unnormalized), q-subtile pairs packed
                for jp in range(2):
                    j2a, j2b = 2 * jp, 2 * jp + 1
                    ya = ps_misc.tile([128, 512], F32, tag="m")
                    yb = ps_misc.tile([128, 512], F32, tag="m")
                    nc.tensor.matmul(
                        ya,
                        u_sb[0:64, j2a * 128 : (j2a + 1) * 128],
                        wo_sb[0:64, :],
                        start=True,
                        stop=True,
                    )
                    nc.tensor.matmul(
                        yb,
                        u2_sb[64:128, j2b * 128 : (j2b + 1) * 128],
                        wo_sb[64:128, :],
                        start=True,
                        stop=True,
                    )
                    for y_ps, j2 in ((ya, j2a), (yb, j2b)):
                        y_sb = ypool.tile([128, 512], F32, tag="y")
                        nc.vector.tensor_copy(y_sb, y_ps)
                        r0 = row0 + j2 * 128
                        nc.sync.dma_start(out=y_d.ap()[r0 : r0 + 128, :], in_=y_sb)

            # Pipeline: proj(tb) immediately enables attn(qb=tb).
            for b in range(B):
                for tb in range(8):
                    proj_block(b, tb)
                    attn_qblock(b, tb)

    nc.compile()
    return nc


def _prep_inputs(x, Wq, bq, Wk, bk, Wv, bv, Wo, bo):
    import ml_dtypes

    xt = np.ascontiguousarray(x.reshape(TOK, D).T).astype(np.float32)
    mask = np.zeros((128, 4, 512), dtype=np.float32)
    p = np.arange(128)[:, None]
    c = np.arange(512)[None, :]
    for d in range(4):
        mask[:, d, :] = (p + 128 * d <= c).astype(np.float32)
    mask = mask.astype(ml_dtypes.bfloat16)
    identb = np.eye(64, dtype=np.float32).astype(ml_dtypes.bfloat16)
    onesb = np.ones((128, NKT), dtype=np.float32).astype(ml_dtypes.bfloat16)

    in_maps = []
    for h in range(H):
        hs = slice(h * HD, (h + 1) * HD)
        wo_h = np.ascontiguousarray(Wo[hs, :]).astype(np.float32)
        in_maps.append(
            {
                "xt": xt,
                "wqk": np.ascontiguousarray(
                    np.concatenate([Wq[:, hs], Wk[:, hs]], axis=1)
                ).astype(np.float32),
                "wv": np.ascontiguousarray(Wv[:, hs]).astype(np.float32),
                "wo": np.concatenate([wo_h, wo_h], axis=0),
                "bqk": np.concatenate([bq[hs], bk[hs]]).reshape(128, 1).astype(
                    np.float32
                ),
                "bv": bv[hs].reshape(HD, 1).astype(np.float32),
                "mask": mask,
                "identb": identb,
                "onesb": onesb,
            }
        )
    return in_maps


def _install_ntff_hook():
    """Register the axon NTFF profiling hook (test-only plumbing)."""
    import types

    try:
        from antenv.axon_hooks import set_axon_ntff_profile_hook  # noqa: F401
    except ImportError:
        m = types.ModuleType("antenv.axon_hooks")
        m._HOOK = None
        m.set_axon_ntff_profile_hook = lambda h: setattr(m, "_HOOK", h)
        m.get_axon_ntff_profile_hook = lambda: m._HOOK
        sys.modules["antenv.axon_hooks"] = m
        import antenv

        antenv.axon_hooks = m
    from antenv.axon_hooks import (
        get_axon_ntff_profile_hook,
        set_axon_ntff_profile_hook,
    )

    if get_axon_ntff_profile_hook() is None:
        import trn_agent_boot.trn_boot as tb

        set_axon_ntff_profile_hook(
            tb._ntff_profile_via_ctypes("/opt/axon/libaxon_pjrt.so")
        )


def kernel(x, Wq, bq, Wk, bk, Wv, bv, Wo, bo, _trace=False):
    x, Wq, bq, Wk, bk, Wv, bv, Wo, bo = (
        np.asarray(a, dtype=np.float32) for a in (x, Wq, bq, Wk, bk, Wv, bv, Wo, bo)
    )
    if "nc" not in _CACHE:
        _CACHE["nc"] = _build()
    nc = _CACHE["nc"]
    in_maps = _prep_inputs(x, Wq, bq, Wk, bk, Wv, bv, Wo, bo)
    kwargs = {}
    if _trace:
        _install_ntff_hook()
        kwargs = dict(trace=True, trace_cores=[0])
    res = run_bass_kernel_spmd(nc, in_maps, core_ids=list(range(8)), **kwargs)
    _CACHE["last_result"] = res
    y = np.zeros((TOK, D), dtype=np.float64)
    for r in res.results:
        y += r["y"].astype(np.float64) / r["l"].astype(np.float64)[:, None]
    y += bo[None, :]
    return y.astype(np.float32).reshape(B, S, D)
